# revision 8
# baseline (speedup 1.0000x reference)
import numpy as np

B, N, CIN, H, UNITS = 8, 2048, 256, 256, 256
NT = N // 128
HT = H // 128
CT = CIN // 128
HALF = NT // 2
SOFTMAX_SHIFT = -110.0

_CACHE = {}


def _build_nc():
    from contextlib import ExitStack

    import concourse.mybir as mybir
    import concourse.tile as tile
    from concourse import bacc
    from concourse.bass import ts
    from concourse.masks import make_identity

    dt = mybir.dt
    AF = mybir.ActivationFunctionType

    nc = bacc.Bacc("TRN2", target_bir_lowering=False, debug=False, num_devices=B)

    x_d = nc.dram_tensor("x", [N, CIN], dt.float32, kind="ExternalInput")
    wq_d = nc.dram_tensor("wq", [CIN, H], dt.float32, kind="ExternalInput")
    bq_d = nc.dram_tensor("bq", [H], dt.float32, kind="ExternalInput")
    wk_d = nc.dram_tensor("wk", [CIN, H], dt.float32, kind="ExternalInput")
    bk_d = nc.dram_tensor("bk", [H], dt.float32, kind="ExternalInput")
    wm_d = nc.dram_tensor("wm", [H, UNITS], dt.float32, kind="ExternalInput")
    bm_d = nc.dram_tensor("bm", [UNITS], dt.float32, kind="ExternalInput")
    y_d = nc.dram_tensor("y", [N, UNITS], dt.float32, kind="ExternalOutput")

    with tile.TileContext(nc) as tc, ExitStack() as ctx:
        const = ctx.enter_context(tc.tile_pool(name="const", bufs=1))
        sb_in = ctx.enter_context(tc.tile_pool(name="sb_in", bufs=6))
        sb_out = ctx.enter_context(tc.tile_pool(name="sb_out", bufs=3))
        e_pool = ctx.enter_context(tc.tile_pool(name="e", bufs=10))
        zs_pool = ctx.enter_context(tc.tile_pool(name="zs", bufs=10))
        st_pool = ctx.enter_context(tc.tile_pool(name="st", bufs=6))
        ps_big = ctx.enter_context(tc.tile_pool(name="ps_big", bufs=3, space="PSUM"))
        ps_sm = ctx.enter_context(tc.tile_pool(name="ps_sm", bufs=2, space="PSUM"))

        ident32 = const.tile([128, 128], dt.float32, tag="ident32")
        make_identity(nc, ident32[:])
        ident16 = const.tile([128, 128], dt.float16, tag="ident16")
        nc.vector.tensor_copy(ident16[:], ident32[:])
        identr = const.tile([128, 128], dt.float32r, tag="identr")
        nc.vector.tensor_copy(identr[:], ident32[:])
        warm_src = const.tile([128, 512], dt.float32, tag="warm_src")
        nc.gpsimd.memset(warm_src[:], 0.0)
        warm_ps = ps_big.tile([128, 512], dt.float32, tag="ps_big", name="warm_ps")
        for wi in range(4):
            nc.tensor.matmul(
                warm_ps[:], ident32[:], warm_src[:],
                start=(wi == 0), stop=(wi == 3),
            )
        ones32 = const.tile([1, 128], dt.float32, tag="ones32")
        nc.gpsimd.memset(ones32[:], 1.0)
        onesr = const.tile([1, 128], dt.float32r, tag="onesr")
        nc.vector.tensor_copy(onesr[:], ones32[:])
        bmr = const.tile([1, UNITS], dt.float32r, tag="bmr")
        nc.gpsimd.dma_start(bmr[:], bm_d[:].unsqueeze(0))
        shift = const.tile([128, 1], dt.float32, tag="shift")
        nc.gpsimd.memset(shift[:], SOFTMAX_SHIFT)

        wq_t, wk_t, wm_t, bq_t, bk_t = [], [], [], [], []
        for ct in range(CT):
            t = const.tile([128, H], dt.float16, tag=f"wq{ct}", name=f"wq{ct}")
            nc.gpsimd.dma_start(t[:], wq_d[ts(ct, 128), :])
            wq_t.append(t)
            t = const.tile([128, H], dt.float16, tag=f"wk{ct}", name=f"wk{ct}")
            nc.gpsimd.dma_start(t[:], wk_d[ts(ct, 128), :])
            wk_t.append(t)
        for ht in range(HT):
            t = const.tile([128, UNITS], dt.float16, tag=f"wm{ht}", name=f"wm{ht}")
            nc.gpsimd.dma_start(t[:], wm_d[ts(ht, 128), :])
            wm_t.append(t)
            t = const.tile([128, 1], dt.float32, tag=f"bq{ht}", name=f"bq{ht}")
            nc.gpsimd.dma_start(t[:], bq_d[ts(ht, 128)].unsqueeze(1))
            bq_t.append(t)
            t = const.tile([128, 1], dt.float32, tag=f"bk{ht}", name=f"bk{ht}")
            nc.gpsimd.dma_start(t[:], bk_d[ts(ht, 128)].unsqueeze(1))
            bk_t.append(t)

        xt = [const.tile([128, N], dt.float16, tag=f"xt{ct}", name=f"xt{ct}") for ct in range(CT)]
        for nt in range(NT):
            xin = sb_in.tile([128, CIN], dt.float16, tag="xin")
            nc.gpsimd.dma_start(xin[:], x_d[ts(nt, 128), :])
            for ct in range(CT):
                pool = ps_sm if (2 * nt + ct) % 2 == 0 else ps_big
                ps = pool.tile([128, 128], dt.float16, tag=pool.name, name="tps")
                nc.tensor.transpose(ps[:], xin[:, ts(ct, 128)], ident16[:])
                if (2 * nt + ct) % 2 == 0:
                    nc.vector.tensor_copy(xt[ct][:, ts(nt, 128)], ps[:])
                else:
                    nc.scalar.copy(xt[ct][:, ts(nt, 128)], ps[:])

        qt = [const.tile([128, N], dt.float16, tag=f"qt{h}", name=f"qt{h}") for h in range(HT)]
        kt = [const.tile([128, N], dt.float16, tag=f"kt{h}", name=f"kt{h}") for h in range(HT)]
        for w_t, b_t, dst in ((wq_t, bq_t, qt), (wk_t, bk_t, kt)):
            for ht in range(HT):
                for sl in range(N // 1024):
                    ps = ps_big.tile([128, 1024], dt.float32, tag="ps_big", name="qkps")
                    for half_sl in range(2):
                        for ct in range(CT):
                            nc.tensor.matmul(
                                ps[:, ts(half_sl, 512)],
                                w_t[ct][:, ts(ht, 128)],
                                xt[ct][:, ts(2 * sl + half_sl, 512)],
                                start=(ct == 0),
                                stop=(ct == CT - 1),
                            )
                    nc.scalar.activation(
                        dst[ht][:, ts(sl, 1024)], ps[:], AF.Relu, bias=b_t[ht][:]
                    )

        z_sb = const.tile([128, NT * UNITS], dt.float32, tag="z")
        for nt in range(NT):
            ps = ps_sm.tile([128, UNITS], dt.float32, tag="ps_sm")
            for ht in range(HT):
                nc.tensor.matmul(
                    ps[:],
                    qt[ht][:, ts(nt, 128)],
                    wm_t[ht][:],
                    start=(ht == 0),
                    stop=(ht == HT - 1),
                )
            nc.vector.tensor_copy(z_sb[:, ts(nt, UNITS)], ps[:])

        partial = const.tile([128, NT * UNITS], dt.float32r, tag="partial")

        def emit_strip(s, e_list, zs_list):
            e = e_pool.tile([128, N], dt.bfloat16, tag="e", name="e")
            for i in range(2):
                sp = ps_big.tile([128, 1024], dt.float32, tag="ps_big", name="sp")
                for sl in range(2):
                    for ht in range(HT):
                        nc.tensor.matmul(
                            sp[:, ts(sl, 512)],
                            kt[ht][:, ts(s, 128)],
                            qt[ht][:, ts(i * 2 + sl, 512)],
                            start=(ht == 0),
                            stop=(ht == HT - 1),
                        )
                nc.scalar.activation(e[:, ts(i, 1024)], sp[:], AF.Exp, bias=shift[:])
            rsum = st_pool.tile([128, 1], dt.float32, tag="rs", name="rsum")
            nc.vector.tensor_reduce(
                rsum[:], e[:], axis=mybir.AxisListType.X, op=mybir.AluOpType.add
            )
            recip = st_pool.tile([128, 1], dt.float32, tag="rs", name="recip")
            nc.vector.reciprocal(recip[:], rsum[:])
            zs = zs_pool.tile([128, UNITS], dt.bfloat16, tag="zs", name="zs")
            nc.vector.tensor_scalar_mul(zs[:], z_sb[:, ts(s, UNITS)], recip[:])
            e_list.append(e)
            zs_list.append(zs)

        def emit_mblock(mb, half, e_list, zs_list):
            ops = ps_sm.tile([128, UNITS], dt.float32, tag="ps_sm", name="ops")
            if half == 0:
                nc.tensor.matmul(ops[:], onesr[:], bmr[:], start=True, stop=False)
            else:
                nc.tensor.matmul(
                    ops[:], identr[:], partial[:, ts(mb, UNITS)],
                    start=True, stop=False,
                )
            for s8 in range(HALF):
                nc.tensor.matmul(
                    ops[:],
                    e_list[s8][:, ts(mb, 128)],
                    zs_list[s8][:],
                    start=False,
                    stop=(s8 == HALF - 1),
                )
            if half == 0:
                nc.vector.tensor_copy(partial[:, ts(mb, UNITS)], ops[:])
            else:
                o = sb_out.tile([128, UNITS], dt.float32, tag="o", name="o")
                nc.scalar.activation(o[:], ops[:], AF.Relu)
                nc.sync.dma_start(y_d[ts(mb, 128), :], o[:])

        e0, zs0 = [], []
        e1, zs1 = [], []
        for s in range(HALF):
            emit_strip(s, e0, zs0)
        for s8 in range(HALF):
            emit_strip(HALF + s8, e1, zs1)
            emit_mblock(2 * s8, 0, e0, zs0)
            emit_mblock(2 * s8 + 1, 0, e0, zs0)
        for mb in range(NT):
            emit_mblock(mb, 1, e1, zs1)

    nc.compile()
    return nc


def _get_nc():
    if "nc" not in _CACHE:
        _CACHE["nc"] = _build_nc()
    return _CACHE["nc"]


def kernel(x, Wq, bq, Wk, bk, Wm, bm):
    from concourse.bass_utils import run_bass_kernel_spmd

    x = np.ascontiguousarray(np.asarray(x, dtype=np.float32))
    weights = {
        "wq": np.ascontiguousarray(np.asarray(Wq, dtype=np.float32)),
        "bq": np.ascontiguousarray(np.asarray(bq, dtype=np.float32)),
        "wk": np.ascontiguousarray(np.asarray(Wk, dtype=np.float32)),
        "bk": np.ascontiguousarray(np.asarray(bk, dtype=np.float32)),
        "wm": np.ascontiguousarray(np.asarray(Wm, dtype=np.float32)),
        "bm": np.ascontiguousarray(np.asarray(bm, dtype=np.float32)),
    }
    nc = _get_nc()
    in_maps = [{"x": x[b], **weights} for b in range(B)]
    res = run_bass_kernel_spmd(nc, in_maps, list(range(B)))
    return np.stack([res.results[b]["y"] for b in range(B)], axis=0)


# revision 11
# speedup vs baseline: 1.1303x; 1.1303x over previous
import numpy as np

B, N, CIN, H, UNITS = 8, 2048, 256, 256, 256
NT = N // 128
HT = H // 128
CT = CIN // 128
HALF = NT // 2
SOFTMAX_SHIFT = -110.0

_CACHE = {}


def _build_nc():
    from contextlib import ExitStack

    import concourse.mybir as mybir
    import concourse.tile as tile
    from concourse import bacc
    from concourse.bass import ts
    from concourse.masks import make_identity

    dt = mybir.dt
    AF = mybir.ActivationFunctionType

    nc = bacc.Bacc("TRN2", target_bir_lowering=False, debug=False, num_devices=B)

    x_d = nc.dram_tensor("x", [N, CIN], dt.float32, kind="ExternalInput")
    wq_d = nc.dram_tensor("wq", [CIN, H], dt.float32, kind="ExternalInput")
    bq_d = nc.dram_tensor("bq", [H], dt.float32, kind="ExternalInput")
    wk_d = nc.dram_tensor("wk", [CIN, H], dt.float32, kind="ExternalInput")
    bk_d = nc.dram_tensor("bk", [H], dt.float32, kind="ExternalInput")
    wm_d = nc.dram_tensor("wm", [H, UNITS], dt.float32, kind="ExternalInput")
    bm_d = nc.dram_tensor("bm", [UNITS], dt.float32, kind="ExternalInput")
    y_d = nc.dram_tensor("y", [N, UNITS], dt.float32, kind="ExternalOutput")

    with tile.TileContext(nc) as tc, ExitStack() as ctx:
        const = ctx.enter_context(tc.tile_pool(name="const", bufs=1))
        sb_in = ctx.enter_context(tc.tile_pool(name="sb_in", bufs=6))
        sb_out = ctx.enter_context(tc.tile_pool(name="sb_out", bufs=3))
        e_pool = ctx.enter_context(tc.tile_pool(name="e", bufs=16))
        zs_pool = ctx.enter_context(tc.tile_pool(name="zs", bufs=16))
        st_pool = ctx.enter_context(tc.tile_pool(name="st", bufs=6))
        ps_big = ctx.enter_context(tc.tile_pool(name="ps_big", bufs=3, space="PSUM"))
        ps_sm = ctx.enter_context(tc.tile_pool(name="ps_sm", bufs=2, space="PSUM"))

        ident32 = const.tile([128, 128], dt.float32, tag="ident32")
        make_identity(nc, ident32[:])
        ident16 = const.tile([128, 128], dt.float16, tag="ident16")
        nc.vector.tensor_copy(ident16[:], ident32[:])
        warm_src = const.tile([128, 512], dt.float32, tag="warm_src")
        nc.gpsimd.memset(warm_src[:], 0.0)
        warm_ps = ps_big.tile([128, 512], dt.float32, tag="ps_big", name="warm_ps")
        for wi in range(4):
            nc.tensor.matmul(
                warm_ps[:], ident32[:], warm_src[:],
                start=(wi == 0), stop=(wi == 3),
            )
        bm_t = []
        for ut in range(UNITS // 128):
            t = const.tile([128, 1], dt.float32, tag=f"bm{ut}", name=f"bm{ut}")
            nc.gpsimd.dma_start(t[:], bm_d[ts(ut, 128)].unsqueeze(1))
            bm_t.append(t)
        shift = const.tile([128, 1], dt.float32, tag="shift")
        nc.gpsimd.memset(shift[:], SOFTMAX_SHIFT)

        wq_t, wk_t, wm_t, bq_t, bk_t = [], [], [], [], []
        for ct in range(CT):
            t = const.tile([128, H], dt.float16, tag=f"wq{ct}", name=f"wq{ct}")
            nc.gpsimd.dma_start(t[:], wq_d[ts(ct, 128), :])
            wq_t.append(t)
            t = const.tile([128, H], dt.float16, tag=f"wk{ct}", name=f"wk{ct}")
            nc.gpsimd.dma_start(t[:], wk_d[ts(ct, 128), :])
            wk_t.append(t)
        for ht in range(HT):
            t = const.tile([128, UNITS], dt.float16, tag=f"wm{ht}", name=f"wm{ht}")
            nc.gpsimd.dma_start(t[:], wm_d[ts(ht, 128), :])
            wm_t.append(t)
            t = const.tile([128, 1], dt.float32, tag=f"bq{ht}", name=f"bq{ht}")
            nc.gpsimd.dma_start(t[:], bq_d[ts(ht, 128)].unsqueeze(1))
            bq_t.append(t)
            t = const.tile([128, 1], dt.float32, tag=f"bk{ht}", name=f"bk{ht}")
            nc.gpsimd.dma_start(t[:], bk_d[ts(ht, 128)].unsqueeze(1))
            bk_t.append(t)

        xt = [const.tile([128, N], dt.float16, tag=f"xt{ct}", name=f"xt{ct}") for ct in range(CT)]
        for nt in range(NT):
            xin = sb_in.tile([128, CIN], dt.float16, tag="xin")
            nc.gpsimd.dma_start(xin[:], x_d[ts(nt, 128), :])
            for ct in range(CT):
                pool = ps_sm if (2 * nt + ct) % 2 == 0 else ps_big
                ps = pool.tile([128, 128], dt.float16, tag=pool.name, name="tps")
                nc.tensor.transpose(ps[:], xin[:, ts(ct, 128)], ident16[:])
                if (2 * nt + ct) % 2 == 0:
                    nc.vector.tensor_copy(xt[ct][:, ts(nt, 128)], ps[:])
                else:
                    nc.scalar.copy(xt[ct][:, ts(nt, 128)], ps[:])

        qt = [const.tile([128, N], dt.float16, tag=f"qt{h}", name=f"qt{h}") for h in range(HT)]
        kt = [const.tile([128, N], dt.float16, tag=f"kt{h}", name=f"kt{h}") for h in range(HT)]
        for w_t, b_t, dst in ((wq_t, bq_t, qt), (wk_t, bk_t, kt)):
            for ht in range(HT):
                for sl in range(N // 1024):
                    ps = ps_big.tile([128, 1024], dt.float32, tag="ps_big", name="qkps")
                    for half_sl in range(2):
                        for ct in range(CT):
                            nc.tensor.matmul(
                                ps[:, ts(half_sl, 512)],
                                w_t[ct][:, ts(ht, 128)],
                                xt[ct][:, ts(2 * sl + half_sl, 512)],
                                start=(ct == 0),
                                stop=(ct == CT - 1),
                            )
                    nc.scalar.activation(
                        dst[ht][:, ts(sl, 1024)], ps[:], AF.Relu, bias=b_t[ht][:]
                    )

        z_sb = const.tile([128, NT * UNITS], dt.float32, tag="z")
        for nt in range(NT):
            ps = ps_sm.tile([128, UNITS], dt.float32, tag="ps_sm")
            for ht in range(HT):
                nc.tensor.matmul(
                    ps[:],
                    qt[ht][:, ts(nt, 128)],
                    wm_t[ht][:],
                    start=(ht == 0),
                    stop=(ht == HT - 1),
                )
            nc.vector.tensor_copy(z_sb[:, ts(nt, UNITS)], ps[:])

        def emit_strip(s, e_list, zs_list):
            e = e_pool.tile([128, N], dt.bfloat16, tag="e", name="e")
            for i in range(2):
                sp = ps_big.tile([128, 1024], dt.float32, tag="ps_big", name="sp")
                for sl in range(2):
                    for ht in range(HT):
                        nc.tensor.matmul(
                            sp[:, ts(sl, 512)],
                            kt[ht][:, ts(s, 128)],
                            qt[ht][:, ts(i * 2 + sl, 512)],
                            start=(ht == 0),
                            stop=(ht == HT - 1),
                        )
                nc.scalar.activation(e[:, ts(i, 1024)], sp[:], AF.Exp, bias=shift[:])
            rsum = st_pool.tile([128, 1], dt.float32, tag="rs", name="rsum")
            nc.vector.tensor_reduce(
                rsum[:], e[:], axis=mybir.AxisListType.X, op=mybir.AluOpType.add
            )
            recip = st_pool.tile([128, 1], dt.float32, tag="rs", name="recip")
            nc.vector.reciprocal(recip[:], rsum[:])
            zs = zs_pool.tile([128, UNITS], dt.bfloat16, tag="zs", name="zs")
            nc.vector.tensor_scalar_mul(zs[:], z_sb[:, ts(s, UNITS)], recip[:])
            e_list.append(e)
            zs_list.append(zs)

        e_list, zs_list = [], []
        for s in range(NT):
            emit_strip(s, e_list, zs_list)

        y_stage = [
            sb_out.tile([128, UNITS], dt.float32, tag="ystage", name=f"yst{mb}", bufs=16)
            for mb in range(NT)
        ]
        n_done = [0] * NT
        for ut in range(UNITS // 128):
            for mq in range(4):
                ops = ps_big.tile([128, 512], dt.float32, tag="ps_big", name="otps")
                for s8 in range(NT):
                    nc.tensor.matmul(
                        ops[:],
                        zs_list[s8][:, ts(ut, 128)],
                        e_list[s8][:, ts(mq, 512)],
                        start=(s8 == 0),
                        stop=(s8 == NT - 1),
                    )
                ot = sb_out.tile([128, 512], dt.float32, tag="ot", name="ot")
                nc.scalar.activation(ot[:], ops[:], AF.Relu, bias=bm_t[ut][:])
                for blk in range(4):
                    mb = mq * 4 + blk
                    tp = ps_sm.tile([128, 128], dt.float32, tag="ps_sm", name="ytp")
                    nc.tensor.transpose(tp[:], ot[:, ts(blk, 128)], ident32[:])
                    nc.vector.tensor_copy(y_stage[mb][:, ts(ut, 128)], tp[:])
                    n_done[mb] += 1
                    if n_done[mb] == UNITS // 128:
                        nc.sync.dma_start(y_d[ts(mb, 128), :], y_stage[mb][:])

    nc.compile()
    return nc


def _get_nc():
    if "nc" not in _CACHE:
        _CACHE["nc"] = _build_nc()
    return _CACHE["nc"]


def kernel(x, Wq, bq, Wk, bk, Wm, bm):
    from concourse.bass_utils import run_bass_kernel_spmd

    x = np.ascontiguousarray(np.asarray(x, dtype=np.float32))
    weights = {
        "wq": np.ascontiguousarray(np.asarray(Wq, dtype=np.float32)),
        "bq": np.ascontiguousarray(np.asarray(bq, dtype=np.float32)),
        "wk": np.ascontiguousarray(np.asarray(Wk, dtype=np.float32)),
        "bk": np.ascontiguousarray(np.asarray(bk, dtype=np.float32)),
        "wm": np.ascontiguousarray(np.asarray(Wm, dtype=np.float32)),
        "bm": np.ascontiguousarray(np.asarray(bm, dtype=np.float32)),
    }
    nc = _get_nc()
    in_maps = [{"x": x[b], **weights} for b in range(B)]
    res = run_bass_kernel_spmd(nc, in_maps, list(range(B)))
    return np.stack([res.results[b]["y"] for b in range(B)], axis=0)
